# revision 1
# baseline (speedup 1.0000x reference)
"""Bass/Tile Trainium2 kernel for masked dot-product attention.

Problem: Q,K,V [2,16,2048,64] f32, attn_mask [2,1,2048,2048] bool (True = masked).
out = softmax(QK^T/8 masked) @ V, computed on 8 NeuronCores, batch*head sharded
(4 heads per core, each core's heads all in one batch so it needs one mask).

Device-side layout ("layout B" — transposed scores so no on-device attn transposes):
  sT[k, q]   = K @ Q^T        (bf16 matmuls, k on partitions, 128-k tiles)
  eT[k, q]   = exp(sT / 8)    (ScalarE; max-subtraction skipped: |s/8| <~ 5.5)
  eT        *= keepT[k, q]    (VectorE, bf16; keep = ~mask as 0/1, host-transposed)
  outT[d, q] = sum_k Vp[k, d] * eT[k, q]   (TensorE, Vp has a ones column ->
               row 64 of outT = softmax denominators)
  finalize per 512-q chunk: DVE recip of sums row + gpsimd partition
  broadcast + DVE mul; output stored [d, q], host transposes to [q, d].
"""

import numpy as np
import ml_dtypes

B, H, S, DK = 2, 16, 2048, 64
NCORES = 8
HPC = (B * H) // NCORES  # heads per core = 4
KT = S // 128            # 16 k-tiles
VPW = 72                 # v-block stride (64 v + 1 ones + pad to 16B align)
BF16 = ml_dtypes.bfloat16

_CACHE = {}


def _build(hpc=HPC, compile=True, reps=1,
           stages=("qk", "exp", "mask", "av", "tail"), av_mode="lag3",
           esplit=False, wmask=False, iobufs=2, etail=False):
    import contextlib
    import concourse.bass as bass
    import concourse.mybir as mybir
    import concourse.tile as tile
    from concourse import bacc
    HPC = hpc
    stages = set(stages)
    f32 = mybir.dt.float32
    f32r = mybir.dt.float32r
    bf16 = mybir.dt.bfloat16
    Exp = mybir.ActivationFunctionType.Exp

    nc = bacc.Bacc("TRN2", target_bir_lowering=False, debug=False,
                   num_devices=NCORES)

    qt_d = nc.dram_tensor("qt", [HPC, DK, S], bf16, kind="ExternalInput").ap()
    kt_d = nc.dram_tensor("kt", [HPC, DK, S], bf16, kind="ExternalInput").ap()
    vp_d = nc.dram_tensor("vp", [HPC, 128, KT * VPW], bf16,
                          kind="ExternalInput").ap()
    mk_d = nc.dram_tensor("maskt", [128, KT * S], bf16,
                          kind="ExternalInput").ap()
    out_d = nc.dram_tensor("out", [HPC, DK, S], f32, kind="ExternalOutput").ap()

    with tile.TileContext(nc) as tc:
        with (
            tc.tile_pool(name="const", bufs=1) as const,
            tc.tile_pool(name="io", bufs=iobufs) as io,
            tc.tile_pool(name="epool", bufs=8) as epool,
            tc.tile_pool(name="fin", bufs=4) as fin,
            tc.tile_pool(name="ps_s", bufs=2, space="PSUM") as ps_s,
            tc.tile_pool(name="ps_o", bufs=4, space="PSUM") as ps_o,
        ):
            mask_sb = const.tile([128, KT * S], bf16)
            mask_v = mask_sb.rearrange("p (k q) -> p k q", k=KT)
            mk_v = mk_d.rearrange("p (k q) -> p k q", k=KT)

            # reps>1 wraps the whole body in a hardware loop — used only by
            # the timing harness (wall-clock delta across rep counts)
            loop_ctx = (tc.For_i(0, reps, 1) if reps > 1
                        else contextlib.nullcontext())

            # tail emission for head h is interleaved into head h+1's k-loop
            # (one 512-q chunk per 4 k-iterations) so the next head's QK/exp
            # keep ACT saturated while the tail drains on PE/DVE.
            state = {}

            def emit_tail_chunk(h, qc):
                ots, ost = state[h]
                sl = slice(qc * 512, (qc + 1) * 512)
                ev = fin.tile([65, 512], f32, tag="ev", bufs=8,
                              name=f"ev_h{h}_q{qc}")
                nc.vector.tensor_copy(ev, ots[qc])
                rc1 = fin.tile([1, 512], f32, tag="rc1", bufs=8,
                               name=f"rc1_h{h}_q{qc}")
                nc.vector.reciprocal(rc1, ev[64:65, :])
                bc = fin.tile([DK, 512], f32, tag="bc", bufs=4,
                              name=f"bc_h{h}_q{qc}")
                nc.gpsimd.partition_broadcast(bc, rc1)
                nc.vector.tensor_mul(ost[:, sl], ev[:DK, :], bc)
                # store this 512-q chunk in [d, q] layout (host transposes;
                # gpsimd SWDGE keeps the sync engine free for prefetches)
                nc.gpsimd.dma_start(out=out_d[h][:, sl], in_=ost[:, sl])

            with loop_ctx:
                for h in range(HPC + 1):
                    if h < HPC:
                        qt_sb = io.tile([DK, S], bf16, tag="qt")
                        nc.sync.dma_start(out=qt_sb[:, :1024],
                                          in_=qt_d[h][:, :1024])
                        kt_sb = io.tile([DK, S], bf16, tag="kt")
                        nc.sync.dma_start(out=kt_sb[:, :1024],
                                          in_=kt_d[h][:, :1024])
                        nc.sync.dma_start(out=qt_sb[:, 1024:],
                                          in_=qt_d[h][:, 1024:])
                        nc.sync.dma_start(out=kt_sb[:, 1024:],
                                          in_=kt_d[h][:, 1024:])
                        vp_sb = io.tile([128, KT * VPW], bf16, tag="vp")
                        nc.sync.dma_start(out=vp_sb, in_=vp_d[h])
                        vp_v = vp_sb.rearrange("p (k c) -> p k c", k=KT)  # c=VPW
                        if h == 0:
                            # chunked so the first mask-mul needn't wait for 8MB
                            for k in range(KT):
                                nc.sync.dma_start(out=mask_v[:, k, :],
                                                  in_=mk_v[:, k, :])
                        # fused k-loop: eT[k] = exp(sT/8)*keepT goes straight
                        # into the 4 per-head outT chunk accumulators
                        ots = []
                        for qc in range(4):
                            ot = ps_o.tile([65, 512], f32, tag="ot",
                                           name=f"ot_h{h}_q{qc}")
                            ots.append(ot)
                        ost = io.tile([DK, S], f32, tag="ost",
                                      name=f"ost_h{h}")
                        state[h] = (ots, ost)
                        ets4 = [None] * 8

                    if h == HPC:
                        if "tail" in stages and etail:
                            # phase-ordered epilogue: evacuations first, then
                            # recip+broadcast, then muls — hides the gpsimd
                            # broadcast latency instead of serializing per chunk
                            ots, ost = state[HPC - 1]
                            evs_, rcs_, bcs_ = [], [], []
                            for qc in range(4):
                                ev = fin.tile([65, 512], f32, tag="ev", bufs=8,
                                              name=f"evE_q{qc}")
                                nc.vector.tensor_copy(ev, ots[qc])
                                evs_.append(ev)
                            for qc in range(4):
                                rc1 = fin.tile([1, 512], f32, tag="rc1",
                                               bufs=8, name=f"rc1E_q{qc}")
                                nc.vector.reciprocal(rc1, evs_[qc][64:65, :])
                                bc = fin.tile([DK, 512], f32, tag="bc", bufs=4,
                                              name=f"bcE_q{qc}")
                                nc.gpsimd.partition_broadcast(bc, rc1)
                                bcs_.append(bc)
                            for qc in range(4):
                                sl = slice(qc * 512, (qc + 1) * 512)
                                nc.vector.tensor_mul(ost[:, sl],
                                                     evs_[qc][:DK, :], bcs_[qc])
                                nc.gpsimd.dma_start(out=out_d[HPC - 1][:, sl],
                                                    in_=ost[:, sl])
                        elif "tail" in stages:
                            for qc in range(4):
                                emit_tail_chunk(HPC - 1, qc)
                        break

                    for k in range(KT):
                        if True:
                            et = epool.tile([128, S], bf16, tag="et")
                            ets4[k % 8] = et
                            lhsT = kt_sb[:, k * 128:(k + 1) * 128]
                            for qh in range(2):
                                st = ps_s.tile([128, 1024], f32, tag="st",
                                               name=f"st_h{h}_k{k}_{qh}")
                                if "qk" in stages:
                                    for qc in range(2):
                                        ofs = qh * 1024 + qc * 512
                                        nc.tensor.matmul(
                                            st[:, qc * 512:(qc + 1) * 512],
                                            lhsT,
                                            qt_sb[:, ofs:ofs + 512],
                                            start=True, stop=True)
                                sl = slice(qh * 1024, (qh + 1) * 1024)
                                if "exp" in stages and esplit:
                                    # two half-width exps: the next QK chunk
                                    # only WARs on the first half (subtile
                                    # deps), pipelining the st-slot chain
                                    for eh in range(2):
                                        nc.scalar.activation(
                                            et[:, qh * 1024 + eh * 512:
                                               qh * 1024 + (eh + 1) * 512],
                                            st[:, eh * 512:(eh + 1) * 512],
                                            Exp, scale=1.0 / np.sqrt(DK))
                                elif "exp" in stages:
                                    nc.scalar.activation(
                                        et[:, sl], st, Exp,
                                        scale=1.0 / np.sqrt(DK))
                                if av_mode == "exp150" and qh == 0:
                                    edum = epool.tile([128, 1024], bf16,
                                                      tag="edum", bufs=2,
                                                      name=f"edum_{h}_{k}")
                                    nc.scalar.activation(
                                        edum, st, Exp,
                                        scale=1.0 / np.sqrt(DK))
                                if "mask" in stages and not wmask:
                                    nc.vector.tensor_mul(
                                        et[:, sl], et[:, sl],
                                        mask_v[:, k, sl])
                            if "mask" in stages and wmask:
                                nc.vector.tensor_mul(
                                    et[:, :], et[:, :], mask_v[:, k, :])
                            def emit_av(kk):
                                # lagged so the PE's in-order stream never
                                # stalls on AV's exp/mask dependency ahead
                                # of the next QK group
                                for qc in range(4):
                                    nc.tensor.matmul(
                                        ots[qc], vp_v[:, kk, :65],
                                        ets4[kk % 8][:, qc * 512:(qc + 1) * 512],
                                        start=(kk == 0), stop=(kk == KT - 1))

                            if "av" in stages and av_mode == "acc":
                                for qc in range(4):
                                    nc.tensor.matmul(
                                        ots[qc], vp_v[:, k, :65],
                                        et[:, qc * 512:(qc + 1) * 512],
                                        start=(k == 0), stop=(k == KT - 1))
                            elif "av" in stages and av_mode == "lag2":
                                if k >= 2:
                                    emit_av(k - 2)
                                if k == KT - 1:
                                    emit_av(k - 1)
                                    emit_av(k)
                            elif "av" in stages and av_mode in (
                                    "lag3", "lag3warm"):
                                if k >= 3:
                                    emit_av(k - 3)
                                if av_mode == "lag3warm":
                                    # dummy weight loads fill PE micro-idles
                                    # so the HAM clock gate stays at 8/8
                                    for _ in range(3):
                                        nc.tensor.ldweights(
                                            mask_sb[:, k * 128:(k + 1) * 128])
                                if k == KT - 1:
                                    emit_av(k - 2)
                                    emit_av(k - 1)
                                    emit_av(k)
                            elif "av" in stages and av_mode == "lag5":
                                if k >= 5:
                                    emit_av(k - 5)
                                if k == KT - 1:
                                    for kk in range(k - 4, k + 1):
                                        emit_av(kk)
                        if h > 0 and k % 4 == 3 and "tail" in stages:
                            emit_tail_chunk(h - 1, k // 4)

    if compile:
        nc.compile()
    return nc


def _get_nc():
    if "nc" not in _CACHE:
        _CACHE["nc"] = _build()
    return _CACHE["nc"]


def _shard(Q, K, V, attn_mask):
    """Host-side marshalling: shard/transposes per core."""
    Q = np.asarray(Q, np.float32)
    K = np.asarray(K, np.float32)
    V = np.asarray(V, np.float32)
    attn_mask = np.asarray(attn_mask, bool)

    # keepT[b, p, kt, q] = (~mask)[b, q, kt*128+p], bf16 0/1
    keep = (~attn_mask[:, 0]).astype(BF16)            # [B, q, k]
    mkT = keep.transpose(0, 2, 1)                     # [B, k, q]
    mkT = mkT.reshape(B, KT, 128, S).transpose(0, 2, 1, 3)  # [B, 128, KT, q]
    mkT = np.ascontiguousarray(mkT).reshape(B, 128, KT * S)

    in_maps = []
    for c in range(NCORES):
        b = c // (NCORES // B)
        h0 = (c % (NCORES // B)) * HPC
        QT = np.ascontiguousarray(
            Q[b, h0:h0 + HPC].transpose(0, 2, 1)).astype(BF16)  # [HPC, DK, S]
        KTt = np.ascontiguousarray(
            K[b, h0:h0 + HPC].transpose(0, 2, 1)).astype(BF16)  # [HPC, DK, S]
        vp = np.zeros((HPC, 128, KT, VPW), BF16)
        vp[:, :, :, 64] = 1.0
        vp[:, :, :, :DK] = V[b, h0:h0 + HPC].astype(BF16).reshape(
            HPC, KT, 128, DK).transpose(0, 2, 1, 3)
        in_maps.append({
            "qt": QT,
            "kt": KTt,
            "vp": np.ascontiguousarray(vp).reshape(HPC, 128, KT * VPW),
            "maskt": mkT[b],
        })
    return in_maps


def kernel(Q, K, V, attn_mask):
    from concourse.bass_utils import run_bass_kernel_spmd

    nc = _get_nc()
    in_maps = _shard(Q, K, V, attn_mask)
    res = run_bass_kernel_spmd(nc, in_maps, list(range(NCORES)))
    out = np.empty((B, H, S, DK), np.float32)
    for c in range(NCORES):
        b = c // (NCORES // B)
        h0 = (c % (NCORES // B)) * HPC
        out[b, h0:h0 + HPC] = res.results[c]["out"].transpose(0, 2, 1)
    return out



# revision 6
# speedup vs baseline: 1.8687x; 1.8687x over previous
"""Bass/Tile Trainium2 kernel for masked dot-product attention.

Problem: Q,K,V [2,16,2048,64] f32, attn_mask [2,1,2048,2048] bool (True = masked).
out = softmax(QK^T/8 masked) @ V, computed on 8 NeuronCores, batch*head sharded
(4 heads per core, each core's heads all in one batch so it needs one mask).

Device-side layout ("layout C" — transposed scores, row-tiled QK pairs):
  k-tiles t and t+8 form pair p=t; K^T for tile p sits in SBUF partitions
  0-63, tile p+8 in partitions 64-127 (Q^T is duplicated into both halves).
  Per (pair p, q-chunk qc of 512):
    st2[128, 1024] f32 PSUM (2 banks):
      st2[:, 0:512]    = K_p  @ Q^T chunk   (TensorE tile_position (0,0))
      st2[:, 512:1024] = K_p8 @ Q^T chunk   (TensorE tile_position (64,0))
      -> the two 64-contraction matmuls run CONCURRENTLY in the PE array.
    et2[128, 1024] bf16 = exp(st2 / 8)      (one ScalarE call, N=1024)
    et2 *= keep2[p, qc]                     (one VectorE bf16 mul; keep = ~mask)
    ots[qc][65, 512] += Vp_p^T  @ et2[:, 0:512]     (TensorE, accum in PSUM;
    ots[qc][65, 512] += Vp_p8^T @ et2[:, 512:1024]   row 64 = ones -> denom)
  AV is lagged by `avlag` groups so the in-order PE stream never stalls on
  the exp/mask chain ahead of the next QK pair. ScalarE (exp) is the
  bottleneck engine; everything else hides under it.
  Tail per q-chunk: DVE copy ev<-ots (frees PSUM), reciprocal_approx_fast
  on the denominator row, gpsimd partition-broadcast, bf16 multiply, DMA
  out in [d, q] bf16 (host transposes and casts to f32).
"""

import numpy as np
import ml_dtypes

B, H, S, DK = 2, 16, 2048, 64
NCORES = 8
HPC = (B * H) // NCORES  # heads per core = 4
KT = S // 128            # 16 k-tiles
NP = KT // 2             # 8 pairs
NG = NP * 4              # 32 groups (pair, q-chunk) per head
VPW = 72                 # v-block stride (64 v + 1 ones + pad to 16B align)
BF16 = ml_dtypes.bfloat16

_CACHE = {}


def _build(hpc=HPC, compile=True, reps=1,
           stages=("qk", "exp", "mask", "av", "tail"), avlag=4,
           iobufs=2, ebufs=8, stbufs=2, **_ignored):
    import contextlib
    import concourse.bass as bass
    import concourse.mybir as mybir
    import concourse.tile as tile
    from concourse import bacc
    HPC = hpc
    stages = set(stages)
    f32 = mybir.dt.float32
    bf16 = mybir.dt.bfloat16
    Exp = mybir.ActivationFunctionType.Exp

    nc = bacc.Bacc("TRN2", target_bir_lowering=False, debug=False,
                   num_devices=NCORES)

    qt_d = nc.dram_tensor("qt", [HPC, 128, S], bf16, kind="ExternalInput").ap()
    kt_d = nc.dram_tensor("kt", [HPC, 128, NP * 128], bf16,
                          kind="ExternalInput").ap()
    vp_d = nc.dram_tensor("vp", [HPC, 128, KT * VPW], bf16,
                          kind="ExternalInput").ap()
    mk_d = nc.dram_tensor("maskt", [128, NG * 1024], bf16,
                          kind="ExternalInput").ap()
    out_d = nc.dram_tensor("out", [HPC, DK, S], bf16, kind="ExternalOutput").ap()

    with tile.TileContext(nc) as tc:
        with (
            tc.tile_pool(name="const", bufs=1) as const,
            tc.tile_pool(name="io", bufs=iobufs) as io,
            tc.tile_pool(name="epool", bufs=ebufs) as epool,
            tc.tile_pool(name="fin", bufs=4) as fin,
            tc.tile_pool(name="ps_s", bufs=stbufs, space="PSUM") as ps_s,
            tc.tile_pool(name="ps_o", bufs=4, space="PSUM") as ps_o,
        ):
            mask_sb = const.tile([128, NG * 1024], bf16)
            mask_v = mask_sb.rearrange("p (g q) -> p g q", g=NG)
            mk_v = mk_d.rearrange("p (g q) -> p g q", g=NG)

            loop_ctx = (tc.For_i(0, reps, 1) if reps > 1
                        else contextlib.nullcontext())

            state = {}

            def emit_tail_piece(h, step):
                # row 0 of ots is the denominator (ones column first in vp).
                # step 0-3: evacuate ots -> ev (frees PSUM for next head)
                # step 4-7: reciprocal of denominator row + gpsimd broadcast
                # step 8-11: final multiply + store chunk
                ots, ost, evs, bcs = state[h]
                qc = step % 4
                sl = slice(qc * 512, (qc + 1) * 512)
                if step < 4:
                    ev = fin.tile([65, 512], f32, tag="ev", bufs=8,
                                  name=f"ev_h{h}_q{qc}")
                    nc.vector.tensor_copy(ev, ots[qc])
                    evs[qc] = ev
                elif step < 8:
                    rc1 = fin.tile([1, 512], f32, tag="rc1", bufs=8,
                                   name=f"rc1_h{h}_q{qc}")
                    nc.vector.reciprocal_approx_fast(rc1, evs[qc][0:1, :])
                    bc = fin.tile([65, 512], f32, tag="bc", bufs=4,
                                  name=f"bc_h{h}_q{qc}")
                    nc.gpsimd.partition_broadcast(bc, rc1)
                    bcs[qc] = bc
                else:
                    nc.vector.tensor_mul(ost[:, sl], evs[qc], bcs[qc])
                    nc.gpsimd.dma_start(out=out_d[h][:, sl],
                                        in_=ost[1:65, sl])

            def emit_av(h, g):
                ots, _, _, _ = state[h]
                p, qc = divmod(g, 4)
                et = state[(h, "et")][g % ebufs]
                vp_v = state[(h, "vp")]
                nc.tensor.matmul(ots[qc], vp_v[:, p, :65], et[:, 0:512],
                                 start=(p == 0), stop=False)
                nc.tensor.matmul(ots[qc], vp_v[:, p + NP, :65],
                                 et[:, 512:1024],
                                 start=False, stop=(p == NP - 1))

            with loop_ctx:
                for h in range(HPC + 1):
                    if h < HPC:
                        qt_sb = io.tile([128, S], bf16, tag="qt")
                        nc.sync.dma_start(out=qt_sb[:, :1024],
                                          in_=qt_d[h][:, :1024])
                        kt_sb = io.tile([128, NP * 128], bf16, tag="kt")
                        nc.sync.dma_start(out=kt_sb, in_=kt_d[h])
                        nc.sync.dma_start(out=qt_sb[:, 1024:],
                                          in_=qt_d[h][:, 1024:])
                        vp_sb = io.tile([128, KT * VPW], bf16, tag="vp")
                        nc.sync.dma_start(out=vp_sb, in_=vp_d[h])
                        vp_v = vp_sb.rearrange("p (k c) -> p k c", k=KT)
                        if h == 0:
                            for i in range(8):
                                nc.sync.dma_start(
                                    out=mask_sb[:, i * 4096:(i + 1) * 4096],
                                    in_=mk_d[:, i * 4096:(i + 1) * 4096])
                        ots = []
                        for qc in range(4):
                            ot = ps_o.tile([65, 512], f32, tag="ot",
                                           name=f"ot_h{h}_q{qc}")
                            ots.append(ot)
                        ost = io.tile([65, S], bf16, tag="ost",
                                      name=f"ost_h{h}")
                        state[h] = (ots, ost, [None] * 4, [None] * 4)
                        state[(h, "et")] = [None] * ebufs
                        state[(h, "vp")] = vp_v

                    if h == HPC:
                        # flush the last head's tail
                        if "tail" in stages:
                            state[HPC] = state[HPC - 1]
                            state[(HPC, "et")] = state[(HPC - 1, "et")]
                            state[(HPC, "vp")] = state[(HPC - 1, "vp")]
                            for step in range(12):
                                emit_tail_piece(HPC - 1, step)
                        break

                    for g in range(NG):
                        p, qc = divmod(g, 4)
                        st2 = ps_s.tile([128, 1024], f32, tag="st",
                                        name=f"st_h{h}_g{g}")
                        if "qk" in stages:
                            nc.tensor.matmul(
                                st2[:, 0:512],
                                kt_sb[0:64, p * 128:(p + 1) * 128],
                                qt_sb[0:64, qc * 512:(qc + 1) * 512],
                                start=True, stop=True, tile_position=(0, 0))
                            nc.tensor.matmul(
                                st2[:, 512:1024],
                                kt_sb[64:128, p * 128:(p + 1) * 128],
                                qt_sb[64:128, qc * 512:(qc + 1) * 512],
                                start=True, stop=True, tile_position=(64, 0))
                        et2 = epool.tile([128, 1024], bf16, tag="et",
                                         name=f"et_h{h}_g{g}")
                        state[(h, "et")][g % ebufs] = et2
                        if "exp" in stages:
                            nc.scalar.activation(et2, st2, Exp,
                                                 scale=1.0 / np.sqrt(DK))
                        if "mask" in stages:
                            nc.vector.tensor_mul(et2, et2, mask_v[:, g, :])
                        if "av" in stages:
                            if g >= avlag:
                                emit_av(h, g - avlag)
                            if g == NG - 1:
                                for gg in range(NG - avlag, NG):
                                    emit_av(h, gg)
                        if h > 0 and g < 12 and "tail" in stages:
                            emit_tail_piece(h - 1, g)

    if compile:
        nc.compile()
    return nc


def _get_nc():
    if "nc" not in _CACHE:
        _CACHE["nc"] = _build()
    return _CACHE["nc"]


def _shard(Q, K, V, attn_mask):
    """Host-side marshalling: shard/transposes per core."""
    Q = np.asarray(Q, np.float32)
    K = np.asarray(K, np.float32)
    V = np.asarray(V, np.float32)
    attn_mask = np.asarray(attn_mask, bool)

    # keep2[b][128, g=(p,qc), 1024] = [keepT(tile p) | keepT(tile p+8)] for
    # q columns qc*512:(qc+1)*512, where keepT[b, kp, t, q] = ~mask[b, q, k]
    keep = (~attn_mask[:, 0]).astype(BF16)                   # [B, q, k]
    mkT = keep.transpose(0, 2, 1)                            # [B, k, q]
    mkT = mkT.reshape(B, KT, 128, S)                         # [B, t, kp, q]
    # -> [B, 128, p, qc, half, 512]
    m2 = np.empty((B, 128, NP, 4, 2, 512), BF16)
    for p in range(NP):
        for qc in range(4):
            m2[:, :, p, qc, 0, :] = mkT[:, p, :, qc * 512:(qc + 1) * 512]
            m2[:, :, p, qc, 1, :] = mkT[:, p + NP, :, qc * 512:(qc + 1) * 512]
    m2 = np.ascontiguousarray(m2).reshape(B, 128, NG * 1024)

    in_maps = []
    for c in range(NCORES):
        b = c // (NCORES // B)
        h0 = (c % (NCORES // B)) * HPC
        QT = np.ascontiguousarray(
            Q[b, h0:h0 + HPC].transpose(0, 2, 1)).astype(BF16)  # [HPC, DK, S]
        QT2 = np.concatenate([QT, QT], axis=1)                  # [HPC, 128, S]
        KTt = np.ascontiguousarray(
            K[b, h0:h0 + HPC].transpose(0, 2, 1)).astype(BF16)  # [HPC, DK, S]
        # kt[h, 0:64, p*128:...] = tile p; kt[h, 64:128, ...] = tile p+8
        kt2 = np.empty((HPC, 128, NP * 128), BF16)
        kt2[:, 0:64, :] = KTt[:, :, :NP * 128]
        kt2[:, 64:128, :] = KTt[:, :, NP * 128:]
        vp = np.zeros((HPC, 128, KT, VPW), BF16)
        vp[:, :, :, 0] = 1.0
        vp[:, :, :, 1:DK + 1] = V[b, h0:h0 + HPC].astype(BF16).reshape(
            HPC, KT, 128, DK).transpose(0, 2, 1, 3)
        in_maps.append({
            "qt": QT2,
            "kt": kt2,
            "vp": np.ascontiguousarray(vp).reshape(HPC, 128, KT * VPW),
            "maskt": m2[b],
        })
    return in_maps


def kernel(Q, K, V, attn_mask):
    from concourse.bass_utils import run_bass_kernel_spmd

    nc = _get_nc()
    in_maps = _shard(Q, K, V, attn_mask)
    res = run_bass_kernel_spmd(nc, in_maps, list(range(NCORES)))
    out = np.empty((B, H, S, DK), np.float32)
    for c in range(NCORES):
        b = c // (NCORES // B)
        h0 = (c % (NCORES // B)) * HPC
        out[b, h0:h0 + HPC] = res.results[c]["out"].astype(
            np.float32).transpose(0, 2, 1)
    return out
